# revision 1
# baseline (speedup 1.0000x reference)
"""Trainium2 Bass kernel for nn_Activation_10445360463903.

Pipeline: 2x upsample (12-tap kaiser polyphase FIR) -> LeakyReLU(0.1) ->
2x downsample (12-tap FIR, stride 2).

Strategy (8 NeuronCores, time-sharded data parallel):
  - Host: flatten (8,128,32768)->(1024,32768); shard time into 8 slices of 4096
    with 6-sample edge-clamped halo; TRANSPOSE each shard to [time, ch] and cast
    to bf16 (host-side layout/precision choice). Band filter matrices for the
    polyphase up/down FIRs are built host-side from the filter inputs.
  - Device (per core, all in natural [partition=time | ch] / [ch | time] layouts,
    zero on-device transposes):
      up:   psum_e/o[123, ch] = Wu_e/o.T @ xt_chunk[128, ch]   (TensorE, band as
            stationary operand; chunk stride 123, 5-row overlap)
      act:  ScalarE Lrelu(alpha=0.1) evacuates psum -> bf16 act tiles [123, 2048]
      down: out_psum[ch_grp 128, s] += act_tile.T @ Wd_e/o     (TensorE, act tile
            as stationary operand, band streams; windows of 128 outputs, psum
            groups of 4 windows with 5-col seams resolved at evacuation)
      evac: VectorE copy psum -> SBUF f32, DMA to HBM.
  - Host: gather, fix up the 6 global-edge output columns (the reference
    edge-replicates the *activation* before downsampling, which the device path
    approximates from clamped x; exact values recomputed in numpy).
"""
import os
import numpy as np
import ml_dtypes
from contextlib import ExitStack

import concourse.bass as bass
import concourse.bacc as bacc
import concourse.tile as tile
import concourse.mybir as mybir
from concourse.bass_utils import run_bass_kernel_spmd

# ---------------- problem constants (hardcoded per spec) ----------------
L = 32768
CH = 1024            # flattened batch*channels = 8*128
NCORES = 8
T = L // NCORES      # 4096 outputs per core
HALO = 6
KSIZE = 12
SLOPE = 0.1

STRIDE = 123         # xt chunk stride (chunk height 128, 5-row overlap)
CHUNK = 128
M_UP = 123           # up outputs per chunk per phase
NWIN = 34            # chunks (== down windows) per core
XT_ROWS = STRIDE * (NWIN - 1) + CHUNK    # 4187
WGRP = 4             # down windows per psum tile
WIN_W = 133          # down window width (123 + 10 cross-tile tap columns)
SEAM = 10
PSUM_W = STRIDE * (WGRP - 1) + WIN_W     # 502
NQ = (NWIN + WGRP - 1) // WGRP           # 9

BF16 = mybir.dt.bfloat16
F32 = mybir.dt.float32


# ---------------- host-side helpers ----------------
def _build_bands(up_f, down_f):
    """Band matrices (float64 -> bf16) for the polyphase FIRs."""
    f = np.asarray(up_f, np.float64)
    g = np.asarray(down_f, np.float64)
    wu = np.zeros((CHUNK, 2 * M_UP))
    for m in range(M_UP):
        for a in range(6):
            wu[m + a, m] = 2.0 * f[2 * a]              # Wu_e: cols [0,123)
            wu[m + a, M_UP + m] = 2.0 * f[2 * a + 1]   # Wu_o: cols [123,246)
    wd = np.zeros((M_UP, 2 * WIN_W))
    for k in range(M_UP):
        for n in range(WIN_W):
            a = k - n + 5
            if 0 <= a < 6:
                wd[k, n] = g[2 * a + 1]                # Wde: cols [0,133)
                wd[k, WIN_W + n] = g[2 * a]            # Wdo: cols [133,266)
    return wu.astype(ml_dtypes.bfloat16), wd.astype(ml_dtypes.bfloat16)


def _shard_xt(x_flat, core):
    """x_flat [CH, L] f32 -> [XT_ROWS, CH] bf16, edge-clamped halo + tail pad."""
    t_base = T * core - HALO
    idx = np.clip(np.arange(XT_ROWS) + t_base, 0, L - 1)
    return np.ascontiguousarray(x_flat[:, idx].T).astype(ml_dtypes.bfloat16)


def _edge_fixup(out_flat, x_flat, up_f, down_f):
    """Exact recompute of the 6 global-edge output columns (numpy, float64)."""
    f = np.asarray(up_f, np.float64)
    g = np.asarray(down_f, np.float64)

    def act_at(ms):
        vals = np.zeros((CH, len(ms)))
        for i, m in enumerate(ms):
            t, p = divmod(m, 2)
            acc = np.zeros(CH)
            for a in range(6):
                j = np.clip(t + a - 3 + p, 0, L - 1)
                acc += 2.0 * f[2 * a + p] * x_flat[:, j]
            vals[:, i] = acc
        return np.where(vals >= 0, vals, SLOPE * vals)

    act_lo = act_at(list(range(0, 13)))
    act_hi = act_at(list(range(2 * L - 13, 2 * L)))
    for s in list(range(3)) + list(range(L - 3, L)):
        acc = np.zeros(CH)
        for k in range(KSIZE):
            m = int(np.clip(2 * s - 5 + k, 0, 2 * L - 1))
            acc += g[k] * (act_lo[:, m] if s < 3 else act_hi[:, m - (2 * L - 13)])
        out_flat[:, s] = acc
    return out_flat


# ---------------- device kernel ----------------
def _build_nc():
    nc = bacc.Bacc()
    xt_d = nc.declare_dram_parameter("xt", [XT_ROWS, CH], BF16, isOutput=False)
    wu_d = nc.declare_dram_parameter("wu", [CHUNK, 2 * M_UP], BF16, isOutput=False)
    wd_d = nc.declare_dram_parameter("wd", [M_UP, 2 * WIN_W], BF16, isOutput=False)
    out_d = nc.declare_dram_parameter("out", [CH, T], BF16, isOutput=True)

    with ExitStack() as ctx:
        tc = ctx.enter_context(tile.TileContext(nc))
        wpool = ctx.enter_context(tc.tile_pool(name="w", bufs=1))
        xt_pool = ctx.enter_context(tc.tile_pool(name="xt", bufs=10))
        act_pool = ctx.enter_context(tc.tile_pool(name="act", bufs=12))
        osb_pool = ctx.enter_context(tc.tile_pool(name="osb", bufs=12))
        ups_pool = ctx.enter_context(tc.tile_pool(name="ups", bufs=2, space="PSUM"))
        dps_pool = ctx.enter_context(tc.tile_pool(name="dps", bufs=4, space="PSUM"))

        wu_sb = wpool.tile([CHUNK, 2 * M_UP], BF16, name="wu_sb")
        wd_sb = wpool.tile([M_UP, 2 * WIN_W], BF16, name="wd_sb")

        act_tiles = {}
        seam_sb = {}
        w_loaded = []

        def emit_up(q):
            xt_t = xt_pool.tile([CHUNK, CH], BF16, name=f"xt_{q}", tag="xt")
            nc.sync.dma_start(xt_t[:], xt_d[STRIDE * q: STRIDE * q + CHUNK, :])
            if not w_loaded:
                # filter loads queue AFTER the first (big) chunk load so the
                # critical first-chunk transfer heads the SP DMA queue
                nc.sync.dma_start(wu_sb[:], wu_d[:])
                nc.sync.dma_start(wd_sb[:], wd_d[:])
                w_loaded.append(True)
            a_t = act_pool.tile([M_UP, 2 * CH], BF16, name=f"act_{q}", tag="act")
            for h in (0, 1):
                ups = ups_pool.tile([M_UP, CH], F32, name=f"ups_{q}_{h}", tag="ups")
                nc.tensor.matmul(ups[:, 0:512], wu_sb[:, 0:M_UP],
                                 xt_t[:, 512 * h: 512 * h + 512],
                                 start=True, stop=True)
                nc.tensor.matmul(ups[:, 512:1024], wu_sb[:, M_UP: 2 * M_UP],
                                 xt_t[:, 512 * h: 512 * h + 512],
                                 start=True, stop=True)
                # Lrelu evacuation: [123, 0:512]=even, [512:1024]=odd ->
                # act cols [1024h : 1024h+1024)
                nc.scalar.activation(a_t[:, 1024 * h: 1024 * h + 1024], ups[:],
                                     mybir.ActivationFunctionType.Prelu,
                                     alpha=SLOPE)
            act_tiles[q] = a_t

        def acol(g, stream):
            # act tile col offset for ch-group g (0..7), stream 0=even 1=odd
            h, gg = divmod(g, 4)
            return 1024 * h + 512 * stream + 128 * gg

        for Q in range(NQ):
            qs = [q for q in range(WGRP * Q, min(WGRP * (Q + 1), NWIN))]
            for q in qs:
                emit_up(q)
            s0 = STRIDE * WGRP * Q - 6          # s_loc of psum col 0
            lo = max(0, -s0)
            hi = min(PSUM_W - SEAM, T - s0)
            for g in range(8):
                dps = dps_pool.tile([CHUNK, PSUM_W], F32, name=f"dps_{Q}_{g}",
                                    tag="dps")
                nmm = 2 * len(qs)
                i = 0
                for j, q in enumerate(qs):
                    for stream in (0, 1):
                        c0 = acol(g, stream)
                        nc.tensor.matmul(
                            dps[:, STRIDE * j: STRIDE * j + WIN_W],
                            act_tiles[q][:, c0: c0 + 128],
                            wd_sb[:, WIN_W * stream: WIN_W * stream + WIN_W],
                            start=(i == 0), stop=(i == nmm - 1),
                        )
                        i += 1
                # evacuate full width (incl seam tail cols [492:502), read by
                # the next psum tile's seam add directly from this osb tile).
                # Split copies between DVE and ACT to balance engine load.
                osb = osb_pool.tile([CHUNK, PSUM_W], BF16,
                                    name=f"osb_{Q}_{g}", tag="osb")
                ce = min(PSUM_W, T - s0)        # last cols worth copying
                c0 = SEAM if Q > 0 else lo
                if Q > 0:
                    nc.vector.tensor_add(osb[:, 0:SEAM], dps[:, 0:SEAM],
                                         seam_sb[g][:, 492:502])
                if ce > c0:
                    nc.vector.tensor_copy(osb[:, c0:ce], dps[:, c0:ce])
                seam_sb[g] = osb
                # stores issue from GpSimd (SWDGE) so slow store-waits don't
                # head-of-line block the xt loads on the SP HWDGE queue; the
                # tail groups go via HWDGE (SP is idle by then, and skipping
                # the serial Q7 descriptor emission shortens the drain)
                st_eng = nc.sync if Q >= NQ - 2 else nc.gpsimd
                st_eng.dma_start(
                    out_d[128 * g: 128 * g + 128, s0 + lo: s0 + hi],
                    osb[:, lo:hi])
    nc.finalize()
    return nc


_CACHE = {}


def _get_nc():
    if "nc" not in _CACHE:
        _CACHE["nc"] = _build_nc()
    return _CACHE["nc"]


# ---------------- public entry ----------------
def kernel(x, up_filter, down_filter):
    x = np.asarray(x)
    up_f = np.asarray(up_filter, np.float32)
    down_f = np.asarray(down_filter, np.float32)
    x_flat = np.ascontiguousarray(x.reshape(CH, L))

    wu, wd = _build_bands(up_f, down_f)
    in_maps = []
    for core in range(NCORES):
        in_maps.append({
            "xt": _shard_xt(x_flat, core),
            "wu": wu,
            "wd": wd,
        })

    nc = _get_nc()
    res = run_bass_kernel_spmd(nc, in_maps, core_ids=list(range(NCORES)),
                               trace=bool(os.environ.get("BASS_TRACE")))
    _CACHE["last_results"] = res
    out_flat = np.concatenate([res.results[i]["out"] for i in range(NCORES)],
                              axis=1).astype(np.float64)
    out_flat = _edge_fixup(out_flat, x_flat.astype(np.float64), up_f, down_f)
    return out_flat.reshape(x.shape).astype(np.float32)


if __name__ == "__main__":
    # quick smoke: build the graph only
    nc = _build_nc()
    print("built ok")

